# revision 9
# baseline (speedup 1.0000x reference)
"""CGNL 2D multi-head attention on 8 TRN2 NeuronCores.

v2 (bf16 GEMMs, double-buffered) measured 578us/pass with DVE nearly as
busy as PE (302 vs 332 us in the local timeline sim).  v3 strips DVE:
  - PSUM drains fused with stats: ACT activation(Copy/Square, accum_out=)
    produces u and sum(u), sum(u^2) in the drain itself; DVE
    tensor_tensor_reduce produces the p*g dot in one 256-elem op.
  - big per-chunk reduces and the pgbuf are gone entirely
  - final normalize (2 x 4096-elem ops) moved to the idle GPSIMD engine
Engine budget per pass (sim): PE 332 | ACT ~150 | Pool ~130 | DMA 125 |
DVE ~80.
"""

import numpy as np
import ml_dtypes

import concourse.bass as bass
import concourse.mybir as mybir
import concourse.tile as tile
from concourse import bacc
from concourse.bass_utils import run_bass_kernel_spmd

DIM = 1024
HEADS = 8
H = W = 16
HW = H * W              # 256
B = 128
CG = DIM // HEADS       # 128
SCALE = (DIM // HEADS) ** -0.5
EPS = 1e-5
N_CORES = 8
B_LOC = B // N_CORES    # 16 batches per core
NB = 2                  # batches per chunk (N=512 per matmul)
N_CHUNKS = B_LOC // NB  # 8
KT = DIM // 128         # 8 k-tiles
MT = DIM // 128         # 8 m-tiles (each m-tile == one group)
NRED = CG * HW          # 32768 elements per group-norm group

F32 = mybir.dt.float32
BF16 = mybir.dt.bfloat16
NP_BF16 = ml_dtypes.bfloat16
ACTF = mybir.ActivationFunctionType


def build_bass(timing=False, reps=1, variant="full", hw_loop=False,
               use_ttr=False, use_act_accum=True, use_pool_norm=True):
    nc = bacc.Bacc("TRN2", target_bir_lowering=False, debug=False)

    nch_ext = 1 if timing else N_CHUNKS
    q_d = nc.dram_tensor("q", [nch_ext, 128, KT, NB, HW], BF16, kind="ExternalInput")
    k_d = nc.dram_tensor("k", [nch_ext, 128, KT, NB, HW], BF16, kind="ExternalInput")
    v_d = nc.dram_tensor("v", [nch_ext, 128, KT, NB, HW], BF16, kind="ExternalInput")
    wq_d = nc.dram_tensor("wq", [128, KT, MT, 128], BF16, kind="ExternalInput")
    wk_d = nc.dram_tensor("wk", [128, KT, MT, 128], BF16, kind="ExternalInput")
    wv_d = nc.dram_tensor("wv", [128, KT, MT, 128], BF16, kind="ExternalInput")
    # gs = gamma*SCALE, gb = -gamma*SCALE/NRED, bet = beta; all [p, m]
    gs_d = nc.dram_tensor("gs", [128, MT], F32, kind="ExternalInput")
    gb_d = nc.dram_tensor("gb", [128, MT], F32, kind="ExternalInput")
    bet_d = nc.dram_tensor("bet", [128, MT], F32, kind="ExternalInput")
    if timing:
        out_d = nc.dram_tensor("out_i", [N_CHUNKS, 128, MT, NB, HW], BF16)
        marker_d = nc.dram_tensor("marker", [128, 8], F32, kind="ExternalOutput")
    else:
        out_d = nc.dram_tensor(
            "out", [N_CHUNKS, 128, MT, NB, HW], BF16, kind="ExternalOutput"
        )

    with tile.TileContext(nc) as tc:
        with (
            tc.tile_pool(name="singles", bufs=1) as singles,
            tc.tile_pool(name="xq", bufs=2) as xq_pool,
            tc.tile_pool(name="xk", bufs=2) as xk_pool,
            tc.tile_pool(name="xv", bufs=2) as xv_pool,
            tc.tile_pool(name="usb", bufs=2) as u_pool,
            tc.tile_pool(name="scr", bufs=2) as scr_pool,
            tc.tile_pool(name="stats", bufs=2) as st_pool,
            tc.tile_pool(name="psum_mm", bufs=6, space="PSUM") as psum_mm,
            tc.tile_pool(name="psum_sm", bufs=1, space="PSUM") as psum_sm,
        ):
            # ---- resident weights & constants ----
            wq_sb = singles.tile([128, KT, MT, 128], BF16)
            nc.sync.dma_start(wq_sb[:], wq_d[:])
            wk_sb = singles.tile([128, KT, MT, 128], BF16)
            nc.sync.dma_start(wk_sb[:], wk_d[:])
            wv_sb = singles.tile([128, KT, MT, 128], BF16)
            nc.sync.dma_start(wv_sb[:], wv_d[:])
            gs_sb = singles.tile([128, MT], F32)
            nc.sync.dma_start(gs_sb[:], gs_d[:])
            gb_sb = singles.tile([128, MT], F32)
            nc.sync.dma_start(gb_sb[:], gb_d[:])
            bet_sb = singles.tile([128, MT], F32)
            nc.sync.dma_start(bet_sb[:], bet_d[:])
            ones_sb = singles.tile([128, 1], F32)
            nc.vector.memset(ones_sb[:], 1.0)
            ones_row = singles.tile([1, 128], F32)
            nc.vector.memset(ones_row[:], 1.0)

            from contextlib import ExitStack

            rep_ctx = ExitStack()
            if hw_loop:
                rep_ctx.enter_context(tc.For_i(0, reps, name="reploop"))
                rep_range = [0]
            else:
                rep_range = range(reps)
            for rep in rep_range:
                for c in range(N_CHUNKS):
                    cg = 0 if timing else c
                    # ---- chunk input loads: [p, kt, b, hw], contiguous ----
                    q_sb = xq_pool.tile([128, KT, NB, HW], BF16)
                    k_sb = xk_pool.tile([128, KT, NB, HW], BF16)
                    v_sb = xv_pool.tile([128, KT, NB, HW], BF16)
                    nc.sync.dma_start(q_sb[:], q_d[cg])
                    nc.sync.dma_start(k_sb[:], k_d[cg])
                    nc.sync.dma_start(v_sb[:], v_d[cg])

                    if variant == "dma":
                        u_sb = u_pool.tile([128, MT, NB, HW], BF16)
                        nc.vector.memset(u_sb[:], 0.0)
                        nc.sync.dma_start(out_d[c], u_sb[:])
                        continue

                    u_sb = u_pool.tile([128, MT, NB, HW], BF16)
                    # stat[p, 0..2, m, b] = per-partition [pg, sum_u, ssq_u]
                    stat = st_pool.tile([128, 3, MT, NB], F32)
                    if variant == "gemmx":
                        dump = st_pool.tile([128, MT, 3], F32, tag="dump")

                    for m in range(MT):
                        ps_u = psum_mm.tile([128, NB, HW], F32, tag="mm")
                        for kt in range(KT):
                            nc.tensor.matmul(
                                ps_u[:], wq_sb[:, kt, m, :], q_sb[:, kt, :, :],
                                start=(kt == 0), stop=(kt == KT - 1),
                            )
                        ps_p = psum_mm.tile([128, NB, HW], F32, tag="mm")
                        for kt in range(KT):
                            nc.tensor.matmul(
                                ps_p[:], wk_sb[:, kt, m, :], k_sb[:, kt, :, :],
                                start=(kt == 0), stop=(kt == KT - 1),
                            )
                        ps_g = psum_mm.tile([128, NB, HW], F32, tag="mm")
                        for kt in range(KT):
                            nc.tensor.matmul(
                                ps_g[:], wv_sb[:, kt, m, :], v_sb[:, kt, :, :],
                                start=(kt == 0), stop=(kt == KT - 1),
                            )
                        if variant == "gemmx":
                            # minimal 1-column drains keep PSUM consumers
                            # legal while measuring the pure PE+DMA floor
                            nc.scalar.copy(dump[:, m, 0:1], ps_u[:, 0, 0:1])
                            nc.scalar.copy(dump[:, m, 1:2], ps_p[:, 0, 0:1])
                            nc.scalar.copy(dump[:, m, 2:3], ps_g[:, 0, 0:1])
                            continue
                        if variant == "gemm":
                            continue
                        # fused drains + stats, per (m, b):
                        #   ACT: u_sb = Copy(ps_u), accum -> sum_u
                        #   ACT: scr  = Square(ps_u), accum -> ssq_u
                        #   DVE: p_sb = copy(ps_p); scr2 = p_sb * ps_g,
                        #        accum -> <p,g>   (DVE reads max 1 PSUM ap)
                        sq_scr = scr_pool.tile([128, NB, HW], BF16, tag="sq")
                        pg_scr = scr_pool.tile([128, NB, HW], BF16, tag="pg")
                        p_sb = scr_pool.tile([128, NB, HW], BF16, tag="p")
                        nc.vector.tensor_copy(p_sb[:], ps_p[:])
                        if use_act_accum:
                            for b in range(NB):
                                nc.scalar.activation(
                                    u_sb[:, m, b], ps_u[:, b], ACTF.Copy,
                                    accum_out=stat[:, 1, m, b : b + 1],
                                )
                                # Square reads the bf16 u just drained (SBUF,
                                # half the bytes of re-reading f32 PSUM)
                                nc.scalar.activation(
                                    sq_scr[:, b], u_sb[:, m, b], ACTF.Square,
                                    accum_out=stat[:, 2, m, b : b + 1],
                                )
                        else:
                            nc.scalar.copy(u_sb[:, m], ps_u[:])
                            nc.vector.tensor_mul(
                                sq_scr[:], u_sb[:, m], u_sb[:, m]
                            )
                            nc.vector.tensor_reduce(
                                stat[:, 1, m], u_sb[:, m],
                                axis=mybir.AxisListType.X, op=mybir.AluOpType.add,
                            )
                            nc.vector.tensor_reduce(
                                stat[:, 2, m], sq_scr[:],
                                axis=mybir.AxisListType.X, op=mybir.AluOpType.add,
                            )
                        if use_ttr:
                            for b in range(NB):
                                nc.vector.tensor_tensor_reduce(
                                    out=pg_scr[:, b],
                                    in0=p_sb[:, b], in1=ps_g[:, b],
                                    scale=1.0, scalar=0.0,
                                    op0=mybir.AluOpType.mult,
                                    op1=mybir.AluOpType.add,
                                    accum_out=stat[:, 0, m, b : b + 1],
                                )
                        else:
                            nc.vector.tensor_mul(pg_scr[:], ps_g[:], p_sb[:])
                            nc.vector.tensor_reduce(
                                stat[:, 0, m], pg_scr[:],
                                axis=mybir.AxisListType.X, op=mybir.AluOpType.add,
                            )

                    if variant in ("gemm", "gemmx"):
                        continue

                    # ---- cross-partition totals via ones-matmul ----
                    r_ps = psum_sm.tile([1, 3 * MT * NB], F32, tag="red")
                    nc.tensor.matmul(
                        r_ps[:], ones_sb[:],
                        stat[:].rearrange("p a m b -> p (a m b)"),
                        start=True, stop=True,
                    )
                    r_sb = st_pool.tile([1, 3, MT, NB], F32)
                    nc.vector.tensor_copy(
                        r_sb[:].rearrange("p a m b -> p (a m b)"), r_ps[:]
                    )

                    # ---- per-(m,b) scalar chain on partition 0 ----
                    # D = <p,g>, S = sum u, Q = sum u^2 (totals over group)
                    # A' = D / sqrt((SCALE^2/NRED)*D^2*(Q - S^2/NRED) + eps)
                    # out = u*(A'*gs) + (A'*S*gb + beta)
                    Dv, Sv, Qv = r_sb[:, 0], r_sb[:, 1], r_sb[:, 2]
                    ct = st_pool.tile([1, 4, MT, NB], F32)
                    ab = st_pool.tile([1, 2, MT, NB], F32)
                    nc.vector.tensor_mul(ct[:, 0], Sv, Sv)
                    nc.vector.tensor_scalar(
                        out=ct[:, 1], in0=ct[:, 0], scalar1=-1.0 / NRED,
                        scalar2=None, op0=mybir.AluOpType.mult,
                    )
                    nc.vector.tensor_add(ct[:, 1], ct[:, 1], Qv)         # Q-S^2/N
                    nc.vector.tensor_mul(ct[:, 2], Dv, Dv)               # D^2
                    nc.vector.tensor_mul(ct[:, 2], ct[:, 2], ct[:, 1])
                    nc.vector.tensor_scalar(
                        out=ct[:, 2], in0=ct[:, 2],
                        scalar1=SCALE * SCALE / NRED, scalar2=EPS,
                        op0=mybir.AluOpType.mult, op1=mybir.AluOpType.add,
                    )
                    nc.scalar.sqrt(ct[:, 3], ct[:, 2])
                    nc.vector.reciprocal(ct[:, 3], ct[:, 3])             # r
                    nc.vector.tensor_mul(ab[:, 0], Dv, ct[:, 3])         # A'
                    nc.vector.tensor_mul(ab[:, 1], ab[:, 0], Sv)         # A'*S

                    # ---- broadcast A'|A'S to all partitions (K=1 matmul) ----
                    ab_ps = psum_sm.tile([128, 2 * MT * NB], F32, tag="bc")
                    nc.tensor.matmul(
                        ab_ps[:], ones_row[:],
                        ab[:].rearrange("p a m b -> p (a m b)"),
                        start=True, stop=True,
                    )
                    ab_bc = ab_ps.rearrange("p (a m b) -> p a m b", a=2, b=NB)

                    # ---- scale/bias prep + normalize (gpsimd) ----
                    sc_t = st_pool.tile([128, MT, NB], F32)
                    nc.vector.tensor_mul(
                        sc_t[:], ab_bc[:, 0],
                        gs_sb[:, :, None].to_broadcast((128, MT, NB)),
                    )
                    bi_t = st_pool.tile([128, MT, NB], F32)
                    nc.vector.tensor_mul(
                        bi_t[:], ab_bc[:, 1],
                        gb_sb[:, :, None].to_broadcast((128, MT, NB)),
                    )
                    nc.vector.tensor_add(
                        bi_t[:], bi_t[:],
                        bet_sb[:, :, None].to_broadcast((128, MT, NB)),
                    )
                    # normalize: for fixed (m,b) the scale/bias are
                    # per-partition vectors, so one fused ACT pass each:
                    # u = Identity(u * sc + bi)
                    for m in range(MT):
                        for b in range(NB):
                            nc.scalar.activation(
                                u_sb[:, m, b], u_sb[:, m, b], ACTF.Identity,
                                scale=sc_t[:, m, b : b + 1],
                                bias=bi_t[:, m, b : b + 1],
                            )
                    nc.sync.dma_start(out_d[c], u_sb[:])

            rep_ctx.close()

            if timing:
                mk = singles.tile([128, 8], F32)
                nc.vector.tensor_copy(mk[:], gs_sb[:])
                nc.sync.dma_start(marker_d[:], mk[:])

    nc.compile()
    return nc


_CACHE = {}


def _get_nc():
    if "nc" not in _CACHE:
        _CACHE["nc"] = build_bass()
    return _CACHE["nc"]


def _to_chunk_layout(x, np_dt):
    """(HW, B, C) f32 -> per-core list of [N_CHUNKS, 128, KT, NB, HW]."""
    xt = x.transpose(1, 2, 0)                      # (B, C, HW)
    xt = xt.reshape(B, KT, 128, HW)                # (B, kt, p, hw)
    out = []
    for i in range(N_CORES):
        s = xt[i * B_LOC : (i + 1) * B_LOC]        # (B_LOC, kt, p, hw)
        s = s.reshape(N_CHUNKS, NB, KT, 128, HW).transpose(0, 3, 2, 1, 4)
        out.append(np.ascontiguousarray(s).astype(np_dt))
    return out


def _w_layout(wT, np_dt):
    """(C, D) contraction-major weight -> [128, KT, MT, 128]."""
    return np.ascontiguousarray(
        wT.reshape(KT, 128, MT, 128).transpose(1, 0, 2, 3)
    ).astype(np_dt)


def prep_inputs(inp_q, inp_k, inp_v, Wt, Wp, Wg, Wz, gamma, beta):
    """Host-side prep: layout transform + weight folding. Returns in_maps."""
    qs = _to_chunk_layout(np.asarray(inp_q, np.float32), NP_BF16)
    ks = _to_chunk_layout(np.asarray(inp_k, np.float32), NP_BF16)
    vs = _to_chunk_layout(np.asarray(inp_v, np.float32), NP_BF16)

    # Fold grouped z-conv into theta conv:
    # Wzt[g*CG+d, c] = sum_e Wz[g,d,e] Wt[g*CG+e, c]
    Wt_g = Wt.reshape(HEADS, CG, DIM)
    Wzt = np.einsum(
        "gde,gec->gdc", Wz.astype(np.float64), Wt_g.astype(np.float64)
    )
    Wzt = Wzt.reshape(DIM, DIM).astype(np.float32)

    wq = _w_layout(np.ascontiguousarray(Wzt.T), NP_BF16)
    wk = _w_layout(np.ascontiguousarray(Wp.T), NP_BF16)
    wv = _w_layout(np.ascontiguousarray(Wg.T), NP_BF16)
    gs = np.ascontiguousarray((gamma * SCALE).reshape(MT, 128).T)
    gb = np.ascontiguousarray((-gamma * SCALE / NRED).reshape(MT, 128).T)
    bet = np.ascontiguousarray(beta.reshape(MT, 128).T)

    in_maps = []
    for i in range(N_CORES):
        in_maps.append(
            {
                "q": qs[i], "k": ks[i], "v": vs[i],
                "wq": wq, "wk": wk, "wv": wv,
                "gs": gs, "gb": gb, "bet": bet,
            }
        )
    return in_maps


def run(in_maps, trace=False):
    nc = _get_nc()
    res = run_bass_kernel_spmd(
        nc, in_maps, core_ids=list(range(N_CORES)), trace=trace
    )
    return res


def gather_output(results):
    """Per-core [N_CHUNKS, 128, MT, NB, HW] -> (B, DIM, H, W)."""
    outs = []
    for r in results:
        o = np.asarray(r["out"]).astype(np.float32)  # (chunks, p, m, b, hw)
        o = o.transpose(0, 3, 2, 1, 4).reshape(B_LOC, DIM, HW)
        outs.append(o)
    return np.concatenate(outs, axis=0).reshape(B, DIM, H, W)


def kernel(inp_q, inp_k, inp_v, Wt, Wp, Wg, Wz, gamma, beta):
    in_maps = prep_inputs(inp_q, inp_k, inp_v, Wt, Wp, Wg, Wz, gamma, beta)
    res = run(in_maps)
    return gather_output(res.results)
